# revision 2
# baseline (speedup 1.0000x reference)
"""Trainium2 Bass kernel for patch attention:
    out = softmax(silu(q) @ silu(k)^T * scale, axis=-1)
with q,k: [B=4, H=16, P=1024, D=128] fp32, scale: [1] fp32.

Sharding: B*H = 64 heads split across 8 NeuronCores, 8 heads each.

v2 design (vs the fp32-everywhere baseline at ~134us):

* bf16 on the wire both ways.  Inputs are host-cast to bf16 (halves input
  DMA, enables xbar DMA-transpose); the output is written as bf16 and
  host-upcast to fp32 (halves the dominant output traffic: 32->16 MB/core).
  DMA busy drops from ~105us to ~50us/core and stops being the roofline.
* qT/kT are loaded directly with dma_start_transpose (HBM [P,D] -> SBUF
  [D,P]), eliminating all 128 PE transposes, their PSUM rings and the
  transpose-interleave scheduling of the baseline.
* ACT (scalar engine) is the pipeline clock: per head one tanh over q|k
  [128, 2048] plus four exp ops over m-tile PAIRS [128, 2048] amortize the
  ~352-cycle per-op overhead (8 exps of 1024 would cost 9.2us; 4 of 2048
  cost 8us).  No accum_out: pairing two m-tiles in one exp would mix their
  row sums, so row sums come from one DVE tensor_reduce per pair
  ([128,2,1024] -> [128,2]) instead, where DVE has headroom.
* Normalization splits 6/8 m-tiles on Pool (normalize_recip, f32-in
  bf16-out) and 2/8 on DVE (reciprocal + tensor_scalar_mul) so neither
  engine serializes the epilogue.
* The q-side silu (stt) writes its output permuted (col t*128+j <- row
  8j+t) so score m-tile m holds rows {8j+m}: the output DMA then writes
  4 ADJACENT rows per partition (8KB contiguous bf16) per dma_start,
  2 dma_starts per head.  The k side stays natural.
"""

import numpy as np

B, H, P, D = 4, 16, 1024, 128
N_CORES = 8
G = (B * H) // N_CORES  # heads per core = 8
PT = P // 128  # 8

_cached = {}


def _build_module():
    import concourse.bass as bass
    import concourse.tile as tile
    from concourse import bacc, mybir

    f32 = mybir.dt.float32
    bf16 = mybir.dt.bfloat16
    AF = mybir.ActivationFunctionType

    nc = bacc.Bacc("TRN2", target_bir_lowering=False, debug=False)
    q_d = nc.dram_tensor("q", [G, P, D], bf16, kind="ExternalInput")
    k_d = nc.dram_tensor("k", [G, P, D], bf16, kind="ExternalInput")
    scale_d = nc.dram_tensor("scale", [1], f32, kind="ExternalInput")
    out_d = nc.dram_tensor("out", [G, P, P], bf16, kind="ExternalOutput")

    with tile.TileContext(nc) as tc:
        with (
            tc.tile_pool(name="consts", bufs=1) as consts,
            tc.tile_pool(name="xt", bufs=3) as xtp,
            tc.tile_pool(name="th", bufs=2) as thp,
            tc.tile_pool(name="bt", bufs=3) as btp,
            tc.tile_pool(name="exp", bufs=3) as expp,
            tc.tile_pool(name="outs", bufs=4) as outp,
            tc.tile_pool(name="stats", bufs=8) as statp,
            tc.tile_pool(name="ps", bufs=2, space="PSUM") as psp,
        ):
            xts = {}

            def dma_in(g):
                """Transposed loads: xt[:, 0] = qT, xt[:, 1] = kT (natural)."""
                xt = xtp.tile([128, 2, P], bf16, tag="xt", name=f"xt{g}")
                nc.sync.dma_start_transpose(out=xt[:, 0, :], in_=q_d[g])
                nc.sync.dma_start_transpose(out=xt[:, 1, :], in_=k_d[g])
                xts[g] = xt

            # First input DMAs go before the scale plumbing so the sync
            # engine kicks them immediately.
            dma_in(0)
            dma_in(1)

            scale_sb = consts.tile([128, 1], f32)
            nc.gpsimd.dma_start(out=scale_sb, in_=scale_d[:].to_broadcast([128, 1]))
            # bT = 2*silu => scores are 4x; fold the 1/4 into the exp scale
            scale_adj = consts.tile([128, 1], f32)
            nc.vector.tensor_scalar_mul(scale_adj, scale_sb, 0.25)

            bts = {}

            def emit_silu(g):
                """bT = 2*silu(xT) = (tanh(xT/2)+1)*xT for q|k in one pass.
                The q half is written permuted (col t*128+j <- natural col
                8j+t) so score m-tiles map to stride-8 output rows; the k
                half stays natural (keys in order)."""
                xt = xts.pop(g)
                th = thp.tile([128, 2, P], bf16, tag="th", name=f"th{g}")
                bt = btp.tile([128, 2, P], bf16, tag="bt", name=f"bt{g}")
                nc.scalar.activation(out=th, in_=xt, func=AF.Tanh, scale=0.5)
                # iteration (j, t): natural col 8j+t -> out col t*128+j
                q_nat = [a[:, 0, :].rearrange("d (j t) -> d j t", t=PT)
                         for a in (th, xt)]
                bt_q_v = bt[:, 0, :].rearrange("d (t j) -> d j t", t=PT)
                nc.vector.scalar_tensor_tensor(
                    out=bt_q_v, in0=q_nat[0], scalar=1.0, in1=q_nat[1],
                    op0=mybir.AluOpType.add, op1=mybir.AluOpType.mult,
                )
                nc.vector.scalar_tensor_tensor(
                    out=bt[:, 1, :], in0=th[:, 1, :], scalar=1.0, in1=xt[:, 1, :],
                    op0=mybir.AluOpType.add, op1=mybir.AluOpType.mult,
                )
                bts[g] = bt

            emit_silu(0)

            for g in range(G):
                bt = bts.pop(g)
                btq, btk = bt[:, 0, :], bt[:, 1, :]
                ov = out_d[g].rearrange("(j r) n -> j r n", r=PT)
                for half in range(2):
                    og = outp.tile([128, 4, P], bf16, tag="out",
                                   name=f"out{g}_{half}")
                    for pp in range(2):
                        pair = half * 2 + pp
                        ps = psp.tile([128, 2, P], f32, tag="ps",
                                      name=f"ps{g}_{pair}")
                        for mm in range(2):
                            for h in range(2):
                                nc.tensor.matmul(
                                    ps[:, mm, bass.ts(h, 512)],
                                    btq[:, bass.ts(pair * 2 + mm, 128)],
                                    btk[:, bass.ts(h, 512)],
                                    start=True,
                                    stop=True,
                                )
                        ex = expp.tile([128, 2, P], f32, tag="exp",
                                       name=f"exp{g}_{pair}")
                        sm = statp.tile([128, 2], f32, tag="sum",
                                        name=f"sum{g}_{pair}")
                        nc.scalar.activation(
                            out=ex, in_=ps, func=AF.Exp, scale=scale_adj)
                        nc.vector.tensor_reduce(
                            out=sm, in_=ex, axis=mybir.AxisListType.X,
                            op=mybir.AluOpType.add)
                        if pair < 3:
                            nc.gpsimd.normalize_recip(
                                og[:, pp * 2 + 0, :], ex[:, 0, :], sm[:, 0:1])
                            nc.gpsimd.normalize_recip(
                                og[:, pp * 2 + 1, :], ex[:, 1, :], sm[:, 1:2])
                        else:
                            nc.vector.reciprocal(sm, sm)
                            nc.vector.tensor_scalar_mul(
                                og[:, 2, :], ex[:, 0, :], sm[:, 0:1])
                            nc.vector.tensor_scalar_mul(
                                og[:, 3, :], ex[:, 1, :], sm[:, 1:2])
                        # Lookahead, interleaved at pair boundaries so ACT
                        # never gaps at a head boundary: inputs two heads
                        # out, silu one head out.
                        if pair == 0 and g + 2 < G:
                            dma_in(g + 2)
                        elif pair == 1 and g + 1 < G:
                            emit_silu(g + 1)
                    nc.sync.dma_start(
                        out=ov[:, half * 4:(half + 1) * 4, :], in_=og)

    nc.compile()
    return nc


def _get_nc():
    if "nc" not in _cached:
        _cached["nc"] = _build_module()
    return _cached["nc"]


def kernel(q, k, scale, _trace=False):
    import ml_dtypes
    from concourse.bass_utils import run_bass_kernel_spmd

    nc = _get_nc()
    qf = np.asarray(q, dtype=np.float32).reshape(B * H, P, D).astype(
        ml_dtypes.bfloat16)
    kf = np.asarray(k, dtype=np.float32).reshape(B * H, P, D).astype(
        ml_dtypes.bfloat16)
    sc = np.ascontiguousarray(np.asarray(scale, dtype=np.float32).reshape(1))
    in_maps = [
        {"q": qf[i * G:(i + 1) * G], "k": kf[i * G:(i + 1) * G], "scale": sc}
        for i in range(N_CORES)
    ]
    res = run_bass_kernel_spmd(
        nc, in_maps, core_ids=list(range(N_CORES)), trace=_trace
    )
    out = np.empty((B * H, P, P), dtype=np.float32)
    for i in range(N_CORES):
        out[i * G:(i + 1) * G] = res.results[i]["out"]
    if _trace:
        kernel.last_result = res
    return out.reshape(B, H, P, P)


# revision 6
# speedup vs baseline: 1.0350x; 1.0350x over previous
"""Trainium2 Bass kernel for patch attention:
    out = softmax(silu(q) @ silu(k)^T * scale, axis=-1)
with q,k: [B=4, H=16, P=1024, D=128] fp32, scale: [1] fp32.

Sharding: B*H = 64 heads split across 8 NeuronCores, 8 heads each.

v3 design.  bf16 on the wire both ways (host casts inputs to bf16, upcasts
the bf16 output back to fp32); qT/kT loaded via xbar dma_start_transpose
(no PE transposes).  The engine budget per head (us), measured rates:

  ACT   exp 3 pairs [128,2048] + 2 singles w/ accum + tanh = 10.6  <- clock
  DVE   silu stt 2.3 + quad reduce 4.4 + pair reduce 2.3   =  9.0
  Pool  8x normalize_recip                                 =  9.2
  PE    16 matmul 128x128x512 bf16                         ~  8
  DMA   0.5 MB in + 2 MB out at ~400 GB/s                  ~  6.3

Key constraints that shaped this (from HW traces + the cost model):
* DVE tensor_reduce/pool/stt run 1x only (no perf-mode uops) and stt is
  not a legal Pool-engine opcode, so row sums + stt are the scarce DVE
  resource.  Tiles 0-5 of each head batch exp in [128,2048] pairs (ACT
  per-op overhead amortized; accum_out would mix the two tiles' sums) and
  get row sums from one quad [128,4,1024] + one pair DVE reduce; tiles
  6-7 run unbatched exp WITH accum_out so their sums are free on ACT.
* exp pairs read 4-bank PSUM tiles; 2 buffers = all 8 banks.
* Strided/3D DVE access patterns cost ~2.6x — all DVE ops here are plain
  contiguous; score m-tiles stay in natural row order and the output DMA
  writes 2 KB per (partition, row) chunk instead (512 descriptors/start).
* tanh is in the same ACT table set as exp (one ACT_TABLE_LOAD).
* Normalization runs on Pool (normalize_recip, f32-in bf16-out, recip
  computed internally); the last head's tiles 6-7 normalize on DVE
  (reciprocal + 2x_2p tensor_scalar) to cut the end-of-kernel drain.
"""

import numpy as np

B, H, P, D = 4, 16, 1024, 128
N_CORES = 8
G = (B * H) // N_CORES  # heads per core = 8
PT = P // 128  # 8

_cached = {}


def _build_module():
    import concourse.bass as bass
    import concourse.tile as tile
    from concourse import bacc, mybir

    f32 = mybir.dt.float32
    bf16 = mybir.dt.bfloat16
    AF = mybir.ActivationFunctionType
    ALU = mybir.AluOpType

    nc = bacc.Bacc("TRN2", target_bir_lowering=False, debug=False)
    q_d = nc.dram_tensor("q", [G, P, D], bf16, kind="ExternalInput")
    k_d = nc.dram_tensor("k", [G, P, D], bf16, kind="ExternalInput")
    scale_d = nc.dram_tensor("scale", [1], f32, kind="ExternalInput")
    out_d = nc.dram_tensor("out", [G, P, P], bf16, kind="ExternalOutput")

    with tile.TileContext(nc) as tc:
        with (
            tc.tile_pool(name="consts", bufs=1) as consts,
            tc.tile_pool(name="xt", bufs=3) as xtp,
            tc.tile_pool(name="th", bufs=2) as thp,
            tc.tile_pool(name="bt", bufs=3) as btp,
            tc.tile_pool(name="exp", bufs=2) as expp,
            tc.tile_pool(name="outs", bufs=4) as outp,
            tc.tile_pool(name="stats", bufs=8) as statp,
            tc.tile_pool(name="ps", bufs=2, space="PSUM") as psp,
        ):
            xts = {}

            def dma_in(g):
                """xbar-transposed loads: xt[:, 0] = qT, xt[:, 1] = kT."""
                xt = xtp.tile([128, 2, P], bf16, tag="xt", name=f"xt{g}")
                nc.sync.dma_start_transpose(out=xt[:, 0, :], in_=q_d[g])
                nc.sync.dma_start_transpose(out=xt[:, 1, :], in_=k_d[g])
                xts[g] = xt

            # First input DMAs go before the scale plumbing so the sync
            # engine kicks them immediately.
            dma_in(0)
            dma_in(1)

            scale_sb = consts.tile([128, 1], f32)
            nc.gpsimd.dma_start(out=scale_sb, in_=scale_d[:].to_broadcast([128, 1]))
            # bT = 2*silu => scores are 4x; fold the 1/4 into the exp scale
            scale_adj = consts.tile([128, 1], f32)
            nc.vector.tensor_scalar_mul(scale_adj, scale_sb, 0.25)

            bts = {}

            def emit_silu(g):
                """bT = 2*silu(xT) = (tanh(xT/2)+1)*xT for q|k in one
                tanh (ACT) + one stt (DVE) pass, all contiguous."""
                xt = xts.pop(g)
                th = thp.tile([128, 2, P], bf16, tag="th", name=f"th{g}")
                bt = btp.tile([128, 2, P], bf16, tag="bt", name=f"bt{g}")
                nc.scalar.activation(out=th, in_=xt, func=AF.Tanh, scale=0.5)
                nc.vector.scalar_tensor_tensor(
                    out=bt, in0=th, scalar=1.0, in1=xt,
                    op0=ALU.add, op1=ALU.mult,
                )
                bts[g] = bt

            emit_silu(0)

            def mm_pair(ps, btq, btk, pair):
                """4 matmuls filling one [128, 2, P] PSUM tile with score
                m-tiles (2*pair, 2*pair+1)."""
                for mm in range(2):
                    for h in range(2):
                        nc.tensor.matmul(
                            ps[:, mm, bass.ts(h, 512)],
                            btq[:, bass.ts(pair * 2 + mm, 128)],
                            btk[:, bass.ts(h, 512)],
                            start=True,
                            stop=True,
                        )

            for g in range(G):
                bt = bts.pop(g)
                btq, btk = bt[:, 0, :], bt[:, 1, :]
                # out rows in natural order: row r*128+j -> partition j, slot r
                ov = out_d[g].rearrange("(r j) n -> j r n", j=128)
                last = g == G - 1

                # --- half 0: tiles 0-3, exp pairs, one quad reduce ---
                og = outp.tile([128, 4, P], bf16, tag="out", name=f"out{g}_0")
                ex = expp.tile([128, 4, P], f32, tag="exp", name=f"exp{g}_0")
                sm = statp.tile([128, 4], f32, tag="sum", name=f"sum{g}_0")
                for pp in range(2):
                    ps = psp.tile([128, 2, P], f32, tag="ps",
                                  name=f"ps{g}_0_{pp}")
                    mm_pair(ps, btq, btk, pp)
                    nc.scalar.activation(
                        out=ex[:, pp * 2:pp * 2 + 2, :], in_=ps,
                        func=AF.Exp, scale=scale_adj)
                    # Lookahead interleaved at pair boundaries.
                    if pp == 0 and g + 2 < G:
                        dma_in(g + 2)
                    elif pp == 1 and g + 1 < G:
                        emit_silu(g + 1)
                nc.vector.tensor_reduce(
                    out=sm, in_=ex, axis=mybir.AxisListType.X, op=ALU.add)
                for t in range(4):
                    nc.gpsimd.normalize_recip(
                        og[:, t, :], ex[:, t, :], sm[:, t:t + 1])
                nc.sync.dma_start(out=ov[:, 0:4, :], in_=og)

                # --- half 1: tiles 4-5 as an exp pair (DVE pair reduce),
                # tiles 6-7 unbatched with ACT accum sums ---
                og = outp.tile([128, 4, P], bf16, tag="out", name=f"out{g}_1")
                ex = expp.tile([128, 4, P], f32, tag="exp", name=f"exp{g}_1")
                sm = statp.tile([128, 4], f32, tag="sum", name=f"sum{g}_1")
                ps = psp.tile([128, 2, P], f32, tag="ps", name=f"ps{g}_1_0")
                mm_pair(ps, btq, btk, 2)
                nc.scalar.activation(
                    out=ex[:, 0:2, :], in_=ps, func=AF.Exp, scale=scale_adj)
                nc.vector.tensor_reduce(
                    out=sm[:, 0:2], in_=ex[:, 0:2, :],
                    axis=mybir.AxisListType.X, op=ALU.add)
                ps = psp.tile([128, 2, P], f32, tag="ps", name=f"ps{g}_1_1")
                mm_pair(ps, btq, btk, 3)
                for t in (2, 3):
                    nc.scalar.activation(
                        out=ex[:, t, :], in_=ps[:, t - 2, :], func=AF.Exp,
                        scale=scale_adj, accum_out=sm[:, t:t + 1])
                for t in range(4):
                    if last and t >= 2:
                        # normalize the final tiles on DVE: shorter drain
                        nc.vector.reciprocal(sm[:, t:t + 1], sm[:, t:t + 1])
                        nc.vector.tensor_scalar_mul(
                            og[:, t, :], ex[:, t, :], sm[:, t:t + 1])
                    else:
                        nc.gpsimd.normalize_recip(
                            og[:, t, :], ex[:, t, :], sm[:, t:t + 1])
                nc.sync.dma_start(out=ov[:, 4:8, :], in_=og)

    nc.compile()
    return nc


def _get_nc():
    if "nc" not in _cached:
        _cached["nc"] = _build_module()
    return _cached["nc"]


def kernel(q, k, scale, _trace=False):
    import ml_dtypes
    from concourse.bass_utils import run_bass_kernel_spmd

    nc = _get_nc()
    qf = np.asarray(q, dtype=np.float32).reshape(B * H, P, D).astype(
        ml_dtypes.bfloat16)
    kf = np.asarray(k, dtype=np.float32).reshape(B * H, P, D).astype(
        ml_dtypes.bfloat16)
    sc = np.ascontiguousarray(np.asarray(scale, dtype=np.float32).reshape(1))
    in_maps = [
        {"q": qf[i * G:(i + 1) * G], "k": kf[i * G:(i + 1) * G], "scale": sc}
        for i in range(N_CORES)
    ]
    res = run_bass_kernel_spmd(
        nc, in_maps, core_ids=list(range(N_CORES)), trace=_trace
    )
    out = np.empty((B * H, P, P), dtype=np.float32)
    for i in range(N_CORES):
        out[i * G:(i + 1) * G] = res.results[i]["out"]
    if _trace:
        kernel.last_result = res
    return out.reshape(B, H, P, P)


# revision 13
# speedup vs baseline: 1.2473x; 1.2052x over previous
"""Trainium2 Bass kernel for patch attention:
    out = softmax(silu(q) @ silu(k)^T * scale, axis=-1)
with q,k: [B=4, H=16, P=1024, D=128] fp32, scale: [1] fp32.

Sharding: B*H = 64 heads split across 8 NeuronCores, 8 heads each.

v5 design.  bf16 on the wire both ways (host casts inputs to bf16, upcasts
the bf16 output back to fp32); qT/kT loaded via xbar dma_start_transpose
(no PE transposes).  Per-head engine budget (us), measured rates:

  ACT   8x exp [128,1024] 8.4 + tanh [128,2048] 1.9  = 10.3  <- clock
  Pool  8x normalize_recip [128,1024]                =  8.6
  DVE   silu stt 2.5 + 8 row-sums (ts+accum 2x) 4.8  =  7.3
  PE    8x (ldweights + 2 matmul 128x128x512 bf16)   =  7.5
  DMA   0.5 MB in + 2 MB out at ~400 GB/s            ~  6.3

Design notes, from HW traces + the cost model:
* Row sums do NOT use tensor_reduce (1x-only, 2283ns/pair): they ride a
  tensor_scalar mult-by-1 with accum_out, which keeps the 2x_2p perf mode
  (~0.6us/tile) and dumps the passthrough product into a throwaway bf16
  scratch tile.  The ACT accumulator path is not used in steady state
  (accum_out on batched exps would mix tiles; unbatched exps + read_accum
  would make ACT the bottleneck).
* exp runs per m-tile [128,1024] from a 2-bank PSUM tile, 4 PSUM buffers:
  the exp(t) -> matmul(t+4) -> exp(t+4) chain then has ~2x latency slack,
  so ACT (the clock) never starves on the PSUM handoff.  Batching exp
  pairs [128,2048] saves 0.3us/head of ACT time but needs 4-bank tiles
  (only 2 buffers) whose handoff latency starved ACT for ~1.2us/head.
* Strided/3D DVE access patterns cost ~2.6x, so everything DVE touches is
  contiguous; score m-tiles stay in natural row order and the output DMA
  writes 2 KB per (partition, row) chunk (256 descriptors/start).
* tanh is in the same ACT table set as exp (one ACT_TABLE_LOAD); the
  prologue splits head 0's tanh in half so ACT starts on qT while kT is
  still in flight.  silu(g+1) is emitted mid-head g, after reduces, so
  its stt never head-of-line blocks DVE.
* Concurrent DVE tensor_scalar + Pool normalize_recip contend on SBUF
  ports (a 0.6us mult measured 5us when overlapped with Pool) — so
  normalization is Pool-only and the DVE reduce stream runs well ahead.
* Last head: tiles 6-7 switch to exp with accum_out (sums free on ACT,
  no trailing DVE reduce) so the drain is just 2 Pool normalizes + DMA.
"""

import numpy as np

B, H, P, D = 4, 16, 1024, 128
N_CORES = 8
G = (B * H) // N_CORES  # heads per core = 8
PT = P // 128  # 8

_cached = {}


def _build_module():
    import concourse.bass as bass
    import concourse.tile as tile
    from concourse import bacc, mybir

    f32 = mybir.dt.float32
    bf16 = mybir.dt.bfloat16
    AF = mybir.ActivationFunctionType
    ALU = mybir.AluOpType

    nc = bacc.Bacc("TRN2", target_bir_lowering=False, debug=False)
    q_d = nc.dram_tensor("q", [G, P, D], bf16, kind="ExternalInput")
    k_d = nc.dram_tensor("k", [G, P, D], bf16, kind="ExternalInput")
    scale_d = nc.dram_tensor("scale", [1], f32, kind="ExternalInput")
    out_d = nc.dram_tensor("out", [G, P, P], bf16, kind="ExternalOutput")

    with tile.TileContext(nc) as tc:
        with (
            tc.tile_pool(name="consts", bufs=1) as consts,
            tc.tile_pool(name="xt", bufs=3) as xtp,
            tc.tile_pool(name="th", bufs=2) as thp,
            tc.tile_pool(name="bt", bufs=3) as btp,
            tc.tile_pool(name="exp", bufs=6) as expp,
            tc.tile_pool(name="outs", bufs=6) as outp,
            tc.tile_pool(name="stats", bufs=10) as statp,
            tc.tile_pool(name="ps", bufs=4, space="PSUM") as psp,
        ):
            xts = {}

            def dma_in(g):
                """xbar-transposed loads: xt[:, 0] = qT, xt[:, 1] = kT.
                (All on sync: issuing the k half from nc.scalar races —
                local head 1 came back wrong on 7 of 8 cores.)"""
                xt = xtp.tile([128, 2, P], bf16, tag="xt", name=f"xt{g}")
                nc.sync.dma_start_transpose(out=xt[:, 0, :], in_=q_d[g])
                nc.sync.dma_start_transpose(out=xt[:, 1, :], in_=k_d[g])
                xts[g] = xt

            # First input DMAs go before the scale plumbing so the sync
            # engine kicks them immediately.
            dma_in(0)
            dma_in(1)

            scale_p0 = consts.tile([1, 1], f32)
            nc.sync.dma_start(out=scale_p0, in_=scale_d[:])
            scale_sb = consts.tile([128, 1], f32)
            nc.gpsimd.partition_broadcast(scale_sb, scale_p0)
            # bT = 2*silu => scores are 4x; fold the 1/4 into the exp scale
            scale_adj = consts.tile([128, 1], f32)
            nc.vector.tensor_scalar_mul(scale_adj, scale_sb, 0.25)
            # throwaway target for the reduce-via-tensor_scalar passthrough
            junk = consts.tile([128, P], bf16)

            bts = {}

            def emit_silu(g, split=False):
                """bT = 2*silu(xT) = (tanh(xT/2)+1)*xT for q|k in one
                tanh (ACT) + one stt (DVE) pass, all contiguous.  split=True
                (head 0) runs tanh per half so ACT starts as soon as the q
                transpose lands."""
                xt = xts.pop(g)
                th = thp.tile([128, 2, P], bf16, tag="th", name=f"th{g}")
                bt = btp.tile([128, 2, P], bf16, tag="bt", name=f"bt{g}")
                if split:
                    for i in range(2):
                        nc.scalar.activation(
                            out=th[:, i, :], in_=xt[:, i, :],
                            func=AF.Tanh, scale=0.5)
                else:
                    nc.scalar.activation(out=th, in_=xt, func=AF.Tanh, scale=0.5)
                nc.vector.scalar_tensor_tensor(
                    out=bt, in0=th, scalar=1.0, in1=xt,
                    op0=ALU.add, op1=ALU.mult,
                )
                bts[g] = bt

            emit_silu(0, split=True)

            for g in range(G):
                bt = bts.pop(g)
                btq, btk = bt[:, 0, :], bt[:, 1, :]
                # out rows in natural order: row r*128+j -> partition j, slot r
                ov = out_d[g].rearrange("(r j) n -> j r n", j=128)
                last = g == G - 1

                og = None
                for t in range(PT):
                    ps = psp.tile([128, P], f32, tag="ps", name=f"ps{g}_{t}")
                    for h in range(2):
                        nc.tensor.matmul(
                            ps[:, bass.ts(h, 512)],
                            btq[:, bass.ts(t, 128)],
                            btk[:, bass.ts(h, 512)],
                            start=True,
                            stop=True,
                        )
                    ex = expp.tile([128, P], f32, tag="exp", name=f"exp{g}_{t}")
                    sm = statp.tile([128, 1], f32, tag="sum", name=f"sum{g}_{t}")
                    if last and t >= 6:
                        # drain: sums free on ACT, no trailing DVE reduce
                        nc.scalar.activation(
                            out=ex, in_=ps, func=AF.Exp, scale=scale_adj,
                            accum_out=sm)
                    else:
                        nc.scalar.activation(
                            out=ex, in_=ps, func=AF.Exp, scale=scale_adj)
                        nc.vector.tensor_scalar(
                            out=junk, in0=ex, scalar1=1.0, scalar2=None,
                            op0=ALU.mult, op1=ALU.add, accum_out=sm)
                    if t % 2 == 0:
                        og = outp.tile([128, 2, P], bf16, tag="out",
                                       name=f"out{g}_{t // 2}")
                    nc.gpsimd.normalize_recip(og[:, t % 2, :], ex, sm)
                    if t % 2 == 1:
                        nc.sync.dma_start(
                            out=ov[:, t - 1:t + 1, :], in_=og)
                    # Lookahead: inputs two heads out early in the head;
                    # silu for g+1 mid-head, after this head's first
                    # reduces, so the stt never head-of-line blocks DVE.
                    if t == 0 and g + 2 < G:
                        dma_in(g + 2)
                    elif t == 4 and g + 1 < G:
                        emit_silu(g + 1)

    nc.compile()
    return nc


def _get_nc():
    if "nc" not in _cached:
        _cached["nc"] = _build_module()
    return _cached["nc"]


def kernel(q, k, scale, _trace=False):
    import ml_dtypes
    from concourse.bass_utils import run_bass_kernel_spmd

    nc = _get_nc()
    qf = np.asarray(q, dtype=np.float32).reshape(B * H, P, D).astype(
        ml_dtypes.bfloat16)
    kf = np.asarray(k, dtype=np.float32).reshape(B * H, P, D).astype(
        ml_dtypes.bfloat16)
    sc = np.ascontiguousarray(np.asarray(scale, dtype=np.float32).reshape(1))
    in_maps = [
        {"q": qf[i * G:(i + 1) * G], "k": kf[i * G:(i + 1) * G], "scale": sc}
        for i in range(N_CORES)
    ]
    res = run_bass_kernel_spmd(
        nc, in_maps, core_ids=list(range(N_CORES)), trace=_trace
    )
    out = np.empty((B * H, P, P), dtype=np.float32)
    for i in range(N_CORES):
        out[i * G:(i + 1) * G] = res.results[i]["out"]
    if _trace:
        kernel.last_result = res
    return out.reshape(B, H, P, P)


# revision 14
# speedup vs baseline: 1.3255x; 1.0627x over previous
"""Trainium2 Bass kernel for patch attention:
    out = softmax(silu(q) @ silu(k)^T * scale, axis=-1)
with q,k: [B=4, H=16, P=1024, D=128] fp32, scale: [1] fp32.

Sharding: B*H = 64 heads split across 8 NeuronCores, 8 heads each.

v6 design.  bf16 on the wire both ways (host casts inputs to bf16, upcasts
the bf16 output back to fp32); qT/kT loaded via xbar dma_start_transpose
(no PE transposes).  Per-head engine budget (us), measured rates:

  ACT   exp 3 pairs [128,2048] + 2 singles w/ accum + tanh = 10.6  <- clock
  DVE   silu stt 2.2 + 3 pair row-sum reduces (1x) 6.9     =  9.1
  Pool  8x normalize_recip [128,1024] f32->bf16            =  8.6
  PE    16 matmul 128x128x512 bf16 (+8 ldweights)          ~  8
  DMA   0.5 MB in + 2 MB out at ~400 GB/s                  ~  6.3

Design notes, from HW traces + the cost model:
* Row sums are the scarce resource: every DVE reduction path measures 1x
  (~1.1ns/elem) — tensor_reduce, pool, AND tensor_scalar-with-accum_out
  (the cost model advertises 2x_2p for the latter but silicon runs 1x).
  So tiles 0-5 batch exp in [128,2048] pairs (amortizing ACT's ~350-cycle
  per-op overhead; accum_out there would mix the two tiles' sums) with
  one DVE pair-reduce each, and tiles 6-7 run unbatched exp WITH
  accum_out, making their sums nearly free on ACT (+180ns read_accum).
* exp pairs read 4-bank PSUM tiles; 2 buffers = all 8 banks.  The
  exp(p) -> matmul(p+2) -> exp(p+2) handoff is latency-tight; a burst of
  dummy matmuls during the DMA ramp warms the PE p-state (cold PE runs
  MM 128x128x512 at ~390ns vs ~216ns spec) to soften it.
* Strided/3D DVE access patterns cost ~2.6x, so everything DVE touches is
  contiguous; score m-tiles stay in natural row order and the output DMA
  writes 2 KB per (partition, row) chunk instead (256 descs/start).
* tanh shares the exp ACT table set (one ACT_TABLE_LOAD); head 0's tanh
  is split in half so ACT starts on qT while kT is still in flight.
  silu(g+1) is emitted after head g's reduces so its stt never
  head-of-line blocks DVE.  The scale load is queued on sync BEFORE the
  transposes (it gates the first exp).
* Normalization is Pool-only: concurrent DVE tensor_scalar + Pool
  normalize_recip contend on SBUF ports (a 0.6us DVE mult measured 5us
  when overlapped with Pool normalize).  The last head's output DMAs are
  split per tile-pair so the drain overlaps the final normalizes.
* Issuing the k-half dma_start_transpose from nc.scalar races (local
  head 1 wrong on 7 of 8 cores) — all input DMAs stay on sync.
"""

import numpy as np

B, H, P, D = 4, 16, 1024, 128
N_CORES = 8
G = (B * H) // N_CORES  # heads per core = 8
PT = P // 128  # 8

_cached = {}


def _build_module():
    import concourse.bass as bass
    import concourse.tile as tile
    from concourse import bacc, mybir

    f32 = mybir.dt.float32
    bf16 = mybir.dt.bfloat16
    AF = mybir.ActivationFunctionType
    ALU = mybir.AluOpType

    nc = bacc.Bacc("TRN2", target_bir_lowering=False, debug=False)
    q_d = nc.dram_tensor("q", [G, P, D], bf16, kind="ExternalInput")
    k_d = nc.dram_tensor("k", [G, P, D], bf16, kind="ExternalInput")
    scale_d = nc.dram_tensor("scale", [1], f32, kind="ExternalInput")
    out_d = nc.dram_tensor("out", [G, P, P], bf16, kind="ExternalOutput")

    with tile.TileContext(nc) as tc:
        with (
            tc.tile_pool(name="consts", bufs=1) as consts,
            tc.tile_pool(name="xt", bufs=3) as xtp,
            tc.tile_pool(name="th", bufs=2) as thp,
            tc.tile_pool(name="bt", bufs=3) as btp,
            tc.tile_pool(name="exp", bufs=4) as expp,
            tc.tile_pool(name="outs", bufs=4) as outp,
            tc.tile_pool(name="stats", bufs=8) as statp,
            tc.tile_pool(name="ps", bufs=2, space="PSUM") as psp,
        ):
            xts = {}

            def dma_in(g):
                """xbar-transposed loads: xt[:, 0] = qT, xt[:, 1] = kT."""
                xt = xtp.tile([128, 2, P], bf16, tag="xt", name=f"xt{g}")
                nc.sync.dma_start_transpose(out=xt[:, 0, :], in_=q_d[g])
                nc.sync.dma_start_transpose(out=xt[:, 1, :], in_=k_d[g])
                xts[g] = xt

            # scale first (tiny, gates the first exp), then the first
            # input transposes, all on the sync queue.
            scale_p0 = consts.tile([1, 1], f32)
            nc.sync.dma_start(out=scale_p0, in_=scale_d[:])
            dma_in(0)
            dma_in(1)

            scale_sb = consts.tile([128, 1], f32)
            nc.gpsimd.partition_broadcast(scale_sb, scale_p0)
            # bT = 2*silu => scores are 4x; fold the 1/4 into the exp scale
            scale_adj = consts.tile([128, 1], f32)
            nc.vector.tensor_scalar_mul(scale_adj, scale_sb, 0.25)

            # PE p-state warmup: garbage matmuls while the first input
            # DMAs are in flight (results never read).
            junk = consts.tile([128, 512], bf16)
            nc.vector.memset(junk, 0.0)
            for w in range(3):
                ps = psp.tile([128, 2, P], f32, tag="ps", name=f"warm{w}")
                for mm in range(2):
                    for h in range(2):
                        nc.tensor.matmul(
                            ps[:, mm, bass.ts(h, 512)],
                            junk[:, 0:128],
                            junk,
                            start=True,
                            stop=True,
                        )

            bts = {}

            def emit_silu(g, split=False):
                """bT = 2*silu(xT) = (tanh(xT/2)+1)*xT for q|k in one
                tanh (ACT) + one stt (DVE) pass, all contiguous.  split=True
                (head 0) runs tanh per half so ACT starts as soon as the q
                transpose lands."""
                xt = xts.pop(g)
                th = thp.tile([128, 2, P], bf16, tag="th", name=f"th{g}")
                bt = btp.tile([128, 2, P], bf16, tag="bt", name=f"bt{g}")
                if split:
                    for i in range(2):
                        nc.scalar.activation(
                            out=th[:, i, :], in_=xt[:, i, :],
                            func=AF.Tanh, scale=0.5)
                else:
                    nc.scalar.activation(out=th, in_=xt, func=AF.Tanh, scale=0.5)
                nc.vector.scalar_tensor_tensor(
                    out=bt, in0=th, scalar=1.0, in1=xt,
                    op0=ALU.add, op1=ALU.mult,
                )
                bts[g] = bt

            emit_silu(0, split=True)

            def mm_pair(ps, btq, btk, pair):
                """4 matmuls filling one [128, 2, P] PSUM tile with score
                m-tiles (2*pair, 2*pair+1).  512 moving elements is the ISA
                max per matmul (one PSUM bank)."""
                for mm in range(2):
                    for h in range(2):
                        nc.tensor.matmul(
                            ps[:, mm, bass.ts(h, 512)],
                            btq[:, bass.ts(pair * 2 + mm, 128)],
                            btk[:, bass.ts(h, 512)],
                            start=True,
                            stop=True,
                        )

            for g in range(G):
                bt = bts.pop(g)
                btq, btk = bt[:, 0, :], bt[:, 1, :]
                # out rows in natural order: row r*128+j -> partition j, slot r
                ov = out_d[g].rearrange("(r j) n -> j r n", j=128)
                last = g == G - 1

                for half in range(2):
                    og = outp.tile([128, 4, P], bf16, tag="out",
                                   name=f"out{g}_{half}")
                    for pp in range(2):
                        pair = half * 2 + pp
                        singles = pair == 3  # tiles 6-7: ACT accum sums
                        ps = psp.tile([128, 2, P], f32, tag="ps",
                                      name=f"ps{g}_{pair}")
                        mm_pair(ps, btq, btk, pair)
                        ex = expp.tile([128, 2, P], f32, tag="exp",
                                       name=f"exp{g}_{pair}")
                        sm = statp.tile([128, 2], f32, tag="sum",
                                        name=f"sum{g}_{pair}")
                        if singles:
                            for t in range(2):
                                nc.scalar.activation(
                                    out=ex[:, t, :], in_=ps[:, t, :],
                                    func=AF.Exp, scale=scale_adj,
                                    accum_out=sm[:, t:t + 1])
                        else:
                            nc.scalar.activation(
                                out=ex, in_=ps, func=AF.Exp, scale=scale_adj)
                            nc.vector.tensor_reduce(
                                out=sm, in_=ex,
                                axis=mybir.AxisListType.X, op=ALU.add)
                        for t in range(2):
                            nc.gpsimd.normalize_recip(
                                og[:, pp * 2 + t, :], ex[:, t, :],
                                sm[:, t:t + 1])
                        # Lookahead: inputs two heads out early in the head;
                        # silu for g+1 after this head's DVE reduces so the
                        # stt never head-of-line blocks them.
                        if pair == 0 and g + 2 < G:
                            dma_in(g + 2)
                        elif pair == 2 and g + 1 < G:
                            emit_silu(g + 1)
                        if last:
                            # drain: ship each tile-pair as it completes
                            nc.sync.dma_start(
                                out=ov[:, pair * 2:pair * 2 + 2, :],
                                in_=og[:, pp * 2:pp * 2 + 2, :])
                    if not last:
                        nc.sync.dma_start(
                            out=ov[:, half * 4:(half + 1) * 4, :], in_=og)

    nc.compile()
    return nc


def _get_nc():
    if "nc" not in _cached:
        _cached["nc"] = _build_module()
    return _cached["nc"]


def kernel(q, k, scale, _trace=False):
    import ml_dtypes
    from concourse.bass_utils import run_bass_kernel_spmd

    nc = _get_nc()
    qf = np.asarray(q, dtype=np.float32).reshape(B * H, P, D).astype(
        ml_dtypes.bfloat16)
    kf = np.asarray(k, dtype=np.float32).reshape(B * H, P, D).astype(
        ml_dtypes.bfloat16)
    sc = np.ascontiguousarray(np.asarray(scale, dtype=np.float32).reshape(1))
    in_maps = [
        {"q": qf[i * G:(i + 1) * G], "k": kf[i * G:(i + 1) * G], "scale": sc}
        for i in range(N_CORES)
    ]
    res = run_bass_kernel_spmd(
        nc, in_maps, core_ids=list(range(N_CORES)), trace=_trace
    )
    out = np.empty((B * H, P, P), dtype=np.float32)
    for i in range(N_CORES):
        out[i * G:(i + 1) * G] = res.results[i]["out"]
    if _trace:
        kernel.last_result = res
    return out.reshape(B, H, P, P)


# revision 15
# speedup vs baseline: 1.3530x; 1.0208x over previous
"""Trainium2 Bass kernel for patch attention:
    out = softmax(silu(q) @ silu(k)^T * scale, axis=-1)
with q,k: [B=4, H=16, P=1024, D=128] fp32, scale: [1] fp32.

Sharding: B*H = 64 heads split across 8 NeuronCores, 8 heads each.

v6 design.  bf16 on the wire both ways (host casts inputs to bf16, upcasts
the bf16 output back to fp32); qT/kT loaded via xbar dma_start_transpose
(no PE transposes).  Per-head engine budget (us), measured rates:

  ACT   exp 3 pairs [128,2048] + 2 singles w/ accum + tanh = 10.6  <- clock
  DVE   silu stt 2.2 + 3 pair row-sum reduces (1x) 6.9     =  9.1
  Pool  8x normalize_recip [128,1024] f32->bf16            =  8.6
  PE    16 matmul 128x128x512 bf16 (+8 ldweights)          ~  8
  DMA   0.5 MB in + 2 MB out at ~400 GB/s                  ~  6.3

Design notes, from HW traces + the cost model:
* Row sums are the scarce resource: every DVE reduction path measures 1x
  (~1.1ns/elem) — tensor_reduce, pool, AND tensor_scalar-with-accum_out
  (the cost model advertises 2x_2p for the latter but silicon runs 1x).
  So tiles 0-5 batch exp in [128,2048] pairs (amortizing ACT's ~350-cycle
  per-op overhead; accum_out there would mix the two tiles' sums) with
  one DVE pair-reduce each, and tiles 6-7 run unbatched exp WITH
  accum_out, making their sums nearly free on ACT (+180ns read_accum).
* exp pairs read 4-bank PSUM tiles; 2 buffers = all 8 banks.  The
  exp(p) -> matmul(p+2) -> exp(p+2) handoff is latency-tight; a burst of
  dummy matmuls during the DMA ramp warms the PE p-state (cold PE runs
  MM 128x128x512 at ~390ns vs ~216ns spec) to soften it.
* Strided/3D DVE access patterns cost ~2.6x, so everything DVE touches is
  contiguous; score m-tiles stay in natural row order and the output DMA
  writes 2 KB per (partition, row) chunk instead (256 descs/start).
* tanh shares the exp ACT table set (one ACT_TABLE_LOAD); head 0's tanh
  is split in half so ACT starts on qT while kT is still in flight.
  silu(g+1) is emitted after head g's reduces so its stt never
  head-of-line blocks DVE.  The scale load is queued on sync BEFORE the
  transposes (it gates the first exp).
* Normalization is Pool-only: concurrent DVE tensor_scalar + Pool
  normalize_recip contend on SBUF ports (a 0.6us DVE mult measured 5us
  when overlapped with Pool normalize).  The last head's output DMAs are
  split per tile-pair so the drain overlaps the final normalizes.
* Issuing the k-half dma_start_transpose from nc.scalar races (local
  head 1 wrong on 7 of 8 cores) — all input DMAs stay on sync.
"""

import numpy as np

B, H, P, D = 4, 16, 1024, 128
N_CORES = 8
G = (B * H) // N_CORES  # heads per core = 8
PT = P // 128  # 8

_cached = {}


def _build_module():
    import concourse.bass as bass
    import concourse.tile as tile
    from concourse import bacc, mybir

    f32 = mybir.dt.float32
    bf16 = mybir.dt.bfloat16
    AF = mybir.ActivationFunctionType
    ALU = mybir.AluOpType

    nc = bacc.Bacc("TRN2", target_bir_lowering=False, debug=False)
    q_d = nc.dram_tensor("q", [G, P, D], bf16, kind="ExternalInput")
    k_d = nc.dram_tensor("k", [G, P, D], bf16, kind="ExternalInput")
    scale_d = nc.dram_tensor("scale", [1], f32, kind="ExternalInput")
    out_d = nc.dram_tensor("out", [G, P, P], bf16, kind="ExternalOutput")

    with tile.TileContext(nc) as tc:
        with (
            tc.tile_pool(name="consts", bufs=1) as consts,
            tc.tile_pool(name="xt", bufs=3) as xtp,
            tc.tile_pool(name="th", bufs=2) as thp,
            tc.tile_pool(name="bt", bufs=3) as btp,
            tc.tile_pool(name="exp", bufs=6) as expp,
            tc.tile_pool(name="outs", bufs=4) as outp,
            tc.tile_pool(name="stats", bufs=10) as statp,
            tc.tile_pool(name="ps", bufs=2, space="PSUM") as psp,
        ):
            xts = {}

            def dma_in(g):
                """xbar-transposed loads: xt[:, 0] = qT, xt[:, 1] = kT."""
                xt = xtp.tile([128, 2, P], bf16, tag="xt", name=f"xt{g}")
                nc.sync.dma_start_transpose(out=xt[:, 0, :], in_=q_d[g])
                nc.sync.dma_start_transpose(out=xt[:, 1, :], in_=k_d[g])
                xts[g] = xt

            # scale first (tiny, gates the first exp), then the first
            # input transposes, all on the sync queue.
            scale_p0 = consts.tile([1, 1], f32)
            nc.sync.dma_start(out=scale_p0, in_=scale_d[:])
            dma_in(0)
            dma_in(1)

            scale_sb = consts.tile([128, 1], f32)
            nc.gpsimd.partition_broadcast(scale_sb, scale_p0)
            # bT = 2*silu => scores are 4x; fold the 1/4 into the exp scale
            scale_adj = consts.tile([128, 1], f32)
            nc.vector.tensor_scalar_mul(scale_adj, scale_sb, 0.25)

            # PE p-state warmup: garbage matmuls while the first input
            # DMAs are in flight (results never read).
            junk = consts.tile([128, 512], bf16)
            nc.vector.memset(junk, 0.0)
            for w in range(3):
                ps = psp.tile([128, 2, P], f32, tag="ps", name=f"warm{w}")
                for mm in range(2):
                    for h in range(2):
                        nc.tensor.matmul(
                            ps[:, mm, bass.ts(h, 512)],
                            junk[:, 0:128],
                            junk,
                            start=True,
                            stop=True,
                        )

            bts = {}

            def emit_silu(g, split=False):
                """bT = 2*silu(xT) = (tanh(xT/2)+1)*xT for q|k in one
                tanh (ACT) + one stt (DVE) pass, all contiguous.  split=True
                (head 0) runs tanh per half so ACT starts as soon as the q
                transpose lands."""
                xt = xts.pop(g)
                th = thp.tile([128, 2, P], bf16, tag="th", name=f"th{g}")
                bt = btp.tile([128, 2, P], bf16, tag="bt", name=f"bt{g}")
                if split:
                    for i in range(2):
                        nc.scalar.activation(
                            out=th[:, i, :], in_=xt[:, i, :],
                            func=AF.Tanh, scale=0.5)
                else:
                    nc.scalar.activation(out=th, in_=xt, func=AF.Tanh, scale=0.5)
                nc.vector.scalar_tensor_tensor(
                    out=bt, in0=th, scalar=1.0, in1=xt,
                    op0=ALU.add, op1=ALU.mult,
                )
                bts[g] = bt

            emit_silu(0, split=True)

            def mm_pair(ps, btq, btk, pair):
                """4 matmuls filling one [128, 2, P] PSUM tile with score
                m-tiles (2*pair, 2*pair+1).  512 moving elements is the ISA
                max per matmul (one PSUM bank)."""
                for mm in range(2):
                    for h in range(2):
                        nc.tensor.matmul(
                            ps[:, mm, bass.ts(h, 512)],
                            btq[:, bass.ts(pair * 2 + mm, 128)],
                            btk[:, bass.ts(h, 512)],
                            start=True,
                            stop=True,
                        )

            for g in range(G):
                bt = bts.pop(g)
                btq, btk = bt[:, 0, :], bt[:, 1, :]
                # out rows in natural order: row r*128+j -> partition j, slot r
                ov = out_d[g].rearrange("(r j) n -> j r n", j=128)
                last = g == G - 1

                for half in range(2):
                    og = outp.tile([128, 4, P], bf16, tag="out",
                                   name=f"out{g}_{half}")
                    for pp in range(2):
                        pair = half * 2 + pp
                        singles = pair == 3  # tiles 6-7: ACT accum sums
                        ps = psp.tile([128, 2, P], f32, tag="ps",
                                      name=f"ps{g}_{pair}")
                        mm_pair(ps, btq, btk, pair)
                        ex = expp.tile([128, 2, P], f32, tag="exp",
                                       name=f"exp{g}_{pair}")
                        sm = statp.tile([128, 2], f32, tag="sum",
                                        name=f"sum{g}_{pair}")
                        if singles:
                            for t in range(2):
                                nc.scalar.activation(
                                    out=ex[:, t, :], in_=ps[:, t, :],
                                    func=AF.Exp, scale=scale_adj,
                                    accum_out=sm[:, t:t + 1])
                        else:
                            nc.scalar.activation(
                                out=ex, in_=ps, func=AF.Exp, scale=scale_adj)
                            nc.vector.tensor_reduce(
                                out=sm, in_=ex,
                                axis=mybir.AxisListType.X, op=ALU.add)
                        for t in range(2):
                            if last and t == 1:
                                # drain: odd tiles on DVE so the final
                                # normalizes run Pool/DVE in parallel
                                nc.vector.reciprocal(
                                    sm[:, t:t + 1], sm[:, t:t + 1])
                                nc.vector.tensor_scalar_mul(
                                    og[:, pp * 2 + t, :], ex[:, t, :],
                                    sm[:, t:t + 1])
                            else:
                                nc.gpsimd.normalize_recip(
                                    og[:, pp * 2 + t, :], ex[:, t, :],
                                    sm[:, t:t + 1])
                        # Lookahead: inputs two heads out early in the head;
                        # silu for g+1 after this head's DVE reduces so the
                        # stt never head-of-line blocks them.
                        if pair == 0 and g + 2 < G:
                            dma_in(g + 2)
                        elif pair == 2 and g + 1 < G:
                            emit_silu(g + 1)
                        if last:
                            # drain: ship each tile-pair as it completes
                            nc.sync.dma_start(
                                out=ov[:, pair * 2:pair * 2 + 2, :],
                                in_=og[:, pp * 2:pp * 2 + 2, :])
                    if not last:
                        nc.sync.dma_start(
                            out=ov[:, half * 4:(half + 1) * 4, :], in_=og)

    nc.compile()
    return nc


def _get_nc():
    if "nc" not in _cached:
        _cached["nc"] = _build_module()
    return _cached["nc"]


def kernel(q, k, scale, _trace=False):
    import ml_dtypes
    from concourse.bass_utils import run_bass_kernel_spmd

    nc = _get_nc()
    qf = np.asarray(q, dtype=np.float32).reshape(B * H, P, D).astype(
        ml_dtypes.bfloat16)
    kf = np.asarray(k, dtype=np.float32).reshape(B * H, P, D).astype(
        ml_dtypes.bfloat16)
    sc = np.ascontiguousarray(np.asarray(scale, dtype=np.float32).reshape(1))
    in_maps = [
        {"q": qf[i * G:(i + 1) * G], "k": kf[i * G:(i + 1) * G], "scale": sc}
        for i in range(N_CORES)
    ]
    res = run_bass_kernel_spmd(
        nc, in_maps, core_ids=list(range(N_CORES)), trace=_trace
    )
    out = np.empty((B * H, P, P), dtype=np.float32)
    for i in range(N_CORES):
        out[i * G:(i + 1) * G] = res.results[i]["out"]
    if _trace:
        kernel.last_result = res
    return out.reshape(B, H, P, P)
